# revision 34
# baseline (speedup 1.0000x reference)
"""DigitCaps routing kernel for 8x TRN2 NeuronCores — v2.

Data-parallel over batch (32 b/core). Single contiguous partition-major W
DMA; 16-way tile-packed einsum; u_hat stored (o,n)-major in SBUF so the
per-iteration c-weighting runs as a 2x-mode DVE tensor_tensor with a
stride-0 o-broadcast (no c16 materialization); r-sum via PE matmuls with a
4-stacked-identity stationary; iteration-0 s comes from a U_sum PSUM
accumulation computed during phase 1 (hidden under the W DMA window).
"""

import numpy as np
import ml_dtypes

BF16 = ml_dtypes.bfloat16

B, R, N, C, O = 256, 1152, 10, 32, 16
NCORES = 8
BS = B // NCORES          # 32 batch per core
NG = R // 16              # 72 groups of 16 r's; r = 16G + 4i + j
NO = N * O                # 160
GC = 9                    # routing chunk (G's)
NCH = NG // GC            # 8
SL = 3                    # (g,i)-slices per s-matmul (480 cols)
WCH = 12                  # wt DMA chunk (G's)
NWCH = NG // WCH          # 6
SG = 3                    # phase supergroup (G's per psum round)
EPS = 1e-8

_CACHE = {}


def _build_program(niter=3, reps=1):
    import concourse.bacc as bacc
    import concourse.tile as tile
    import concourse.mybir as mybir
    from contextlib import ExitStack

    f32 = mybir.dt.float32
    bf16 = mybir.dt.bfloat16
    alu = mybir.AluOpType
    act = mybir.ActivationFunctionType
    AX = mybir.AxisListType

    nc = bacc.Bacc("TRN2", target_bir_lowering=False, debug=False)

    # xt[32j+c][G, i, b] = x[b, 16G+4i+j, c]  (j-major partitions)
    xt_d = nc.dram_tensor("xt", [128, NG, 4, BS], bf16, kind="ExternalInput")
    # wt[ck][32j+c][Gck][i][n*16+o] = W[16G+4i+j, n, c, o]  (chunk-major so
    # each chunk is one fully contiguous 2MB HBM read)
    wt_d = nc.dram_tensor(
        "wt", [NWCH, 128, WCH, 4, NO], bf16, kind="ExternalInput"
    )
    i4_d = nc.dram_tensor("i4", [128, BS], bf16, kind="ExternalInput")
    i32_d = nc.dram_tensor("i32", [BS, 128], bf16, kind="ExternalInput")
    ii_d = nc.dram_tensor("ii", [128, 128], bf16, kind="ExternalInput")
    v_d = nc.dram_tensor("v", [BS, NO], f32, kind="ExternalOutput")

    with tile.TileContext(nc) as tc:
        with ExitStack() as ctx:
            persist = ctx.enter_context(tc.tile_pool(name="persist", bufs=1))
            # u_sb[32j+b][g, i, o, n] = u_hat[b, 16g+4i+j, n, o]
            u_sb = persist.tile([128, NG, 4, O, N], bf16, name="u_sb")
            bl = persist.tile([128, NG, 4, N], bf16, name="bl")
            # xts: double-buffered block-diagonal stationary for the einsum.
            # Zeros are written once (off-diagonal never touched again); the
            # four diagonal [32c, 32b] blocks are DMA'd per wt-chunk straight
            # from HBM.
            xts = persist.tile([128, 2, WCH, 4, 128], bf16, name="xts")
            xt_sb = persist.tile([128, NG, 4, BS], bf16, name="xt_sb")
            i4_sb = persist.tile([128, BS], bf16, name="i4_sb")
            i32_sb = persist.tile([BS, 128], bf16, name="i32_sb")
            ii_sb = persist.tile([128, 128], bf16, name="ii_sb")
            nc.vector.memset(xts[:, 0], 0.0)
            nc.gpsimd.memset(xts[:, 1], 0.0)
            nc.sync.dma_start(xt_sb[:], xt_d[:])
            nc.sync.dma_start(i4_sb[:], i4_d[:])
            nc.sync.dma_start(i32_sb[:], i32_d[:])
            nc.sync.dma_start(ii_sb[:], ii_d[:])

            for _rep in range(reps):
                with ExitStack() as rctx:
                    rpool = rctx.enter_context(tc.tile_pool(name="rt", bufs=2))

                    # ---------------- Phase 1: einsum + U_sum ----------------
                    # One matmul per (g, i): block-diagonal x stationary
                    # [128, 128] x full-height wt moving [128, 160] computes
                    # all four j's of u_hat in a single 160-column pass.
                    # U_sum accumulates on the PE (identity stationary) from
                    # evacuated u_sb groups.
                    s0_pool = rctx.enter_context(
                        tc.tile_pool(name="s0ps", bufs=1, space="PSUM")
                    )
                    # S0d[b, n, o] accumulates sum_r u_hat directly from the
                    # (j,c)-contraction of xt (stationary) x wt (moving) —
                    # no dependency on the u_sb evacuation.
                    S0d = s0_pool.tile([BS, N, O], f32, name="S0d", tag="S0d")
                    ns0 = 0
                    with ExitStack() as pctx:
                        wt_pool = pctx.enter_context(
                            tc.tile_pool(name="wt", bufs=2)
                        )
                        ps_pool = pctx.enter_context(
                            tc.tile_pool(name="ps", bufs=1, space="PSUM")
                        )
                        # HAM warm-up: ~3.5us of junk matmuls so the phase
                        # (and everything after) runs at K=8/8.
                        warm = ps_pool.tile([BS, BS], f32, name="warm", tag="psw")
                        for w in range(50):
                            nc.tensor.matmul(
                                warm[:],
                                i4_sb[:],
                                i4_sb[:],
                                start=(w == 0),
                                stop=(w == 49),
                                skip_group_check=True,
                            )
                        CKS = (WCH,) * NWCH
                        ck0 = 0
                        for cki, ckw in enumerate(CKS):
                            wt_t = wt_pool.tile(
                                [128, WCH, 4, NO], bf16, name="wt_t", tag="wt"
                            )
                            nc.sync.dma_start(wt_t[:, 0:ckw], wt_d[cki])
                            buf = cki % 2
                            for j in range(4):
                                nc.vector.tensor_copy(
                                    xts[
                                        32 * j : 32 * (j + 1),
                                        buf,
                                        0:ckw,
                                        :,
                                        32 * j : 32 * (j + 1),
                                    ],
                                    xt_sb[32 * j : 32 * (j + 1), ck0 : ck0 + ckw],
                                )
                            for sgi in range(ckw // SG):
                                g0 = ck0 + sgi * SG
                                for i in range(4):
                                    psi = ps_pool.tile(
                                        [128, SG, N, O],
                                        f32,
                                        name=f"ps{i}",
                                        tag=f"ps{i}",
                                    )
                                    for s in range(SG):
                                        g = g0 + s
                                        nc.tensor.matmul(
                                            psi[:, s, :, :],
                                            xts[:, buf, g - ck0, i, :],
                                            wt_t[:, g - ck0, i, :],
                                            start=True,
                                            stop=True,
                                            skip_group_check=True,
                                        )
                                    for s in range(SG):
                                        g = g0 + s
                                        nc.tensor.matmul(
                                            S0d[:],
                                            xt_sb[:, g, i, :],
                                            wt_t[:, g - ck0, i, :],
                                            start=(ns0 == 0),
                                            stop=(ns0 == R // 4 - 1),
                                            skip_group_check=True,
                                        )
                                        ns0 += 1
                                    src = psi[:].rearrange("p s n o -> p s o n")
                                    dst = u_sb[:, g0 : g0 + SG, i, :, :]
                                    if i < 2:
                                        nc.scalar.activation(dst, src, act.Copy)
                                    else:
                                        nc.vector.tensor_copy(dst, src)
                            ck0 += ckw

                    ch_pool = rctx.enter_context(tc.tile_pool(name="ch", bufs=2))
                    sps_pool = rctx.enter_context(
                        tc.tile_pool(name="sps", bufs=1, space="PSUM")
                    )
                    u0_pool = rctx.enter_context(
                        tc.tile_pool(name="u0ps", bufs=1, space="PSUM")
                    )
                    agr_pool = rctx.enter_context(
                        tc.tile_pool(name="agrps", bufs=2, space="PSUM")
                    )

                    # ---------------- s0 = 0.1 * U_sum -> v0 ----------------
                    st0 = rpool.tile([BS, N, O], f32, name="st0", tag="st")
                    nc.scalar.activation(st0[:], S0d[:], act.Copy, scale=0.1)

                    def squash_and_vb(st, t):
                        """st: [BS, N, O] f32 SBUF -> v_sb f32; returns (v_sb, vb)"""
                        sq = rpool.tile([BS, N, O], f32, name="sq", tag="sq")
                        nc.scalar.activation(sq[:], st[:], act.Square)
                        ssum = rpool.tile([BS, N], f32, name="ssum", tag="ssum")
                        nc.vector.tensor_reduce(
                            ssum[:], sq[:], axis=AX.X, op=alu.add
                        )
                        d1 = rpool.tile([BS, N], f32, name="d1", tag="d1")
                        nc.vector.tensor_scalar_add(d1[:], ssum[:], 1.0)
                        se = rpool.tile([BS, N], f32, name="se", tag="se")
                        nc.vector.tensor_scalar_add(se[:], ssum[:], EPS)
                        sr = rpool.tile([BS, N], f32, name="sr", tag="sr")
                        nc.scalar.activation(sr[:], se[:], act.Sqrt)
                        den2 = rpool.tile([BS, N], f32, name="den2", tag="den2")
                        nc.vector.tensor_mul(den2[:], d1[:], sr[:])
                        rden = rpool.tile([BS, N], f32, name="rden", tag="rden")
                        nc.vector.reciprocal(rden[:], den2[:])
                        scale = rpool.tile([BS, N], f32, name="scale", tag="scale")
                        nc.vector.tensor_mul(scale[:], ssum[:], rden[:])
                        v_sb = rpool.tile([BS, N, O], f32, name="v_sb", tag="v_sb")
                        scale_b = (
                            scale[:]
                            .rearrange("p n -> p n ()")
                            .broadcast_to([BS, N, O])
                        )
                        nc.vector.tensor_mul(v_sb[:], st[:], scale_b)
                        if t == niter - 1:
                            return v_sb, None
                        vB = rpool.tile([BS, O, N], bf16, name="vB", tag="vB")
                        nc.vector.tensor_copy(
                            vB[:], v_sb[:].rearrange("b n o -> b o n")
                        )
                        vbps = u0_pool.tile([128, O, N], f32, name="vbps", tag="vbps")
                        nc.tensor.matmul(
                            vbps[:], i32_sb[:], vB[:], start=True, stop=True
                        )
                        vb = rpool.tile([128, O, N], bf16, name="vb", tag="vb")
                        nc.scalar.activation(vb[:], vbps[:], act.Copy)
                        return v_sb, vb

                    _, vb = squash_and_vb(st0, 0)

                    # ------------- merged sweeps (iterations 1..niter-1) -----
                    # Per chunk: agreement with v_{t-1} -> bl update -> softmax
                    # -> prod -> s-matmuls. One sweep per remaining iteration.
                    n_sl = GC * 4 // SL  # s-matmuls per chunk (480 cols each)
                    for t in range(1, niter):
                        first = t == 1
                        s_ps = sps_pool.tile(
                            [BS, SL, O, N], f32, name="s_ps", tag="s_ps"
                        )
                        mm_k = 0
                        n_mm = NCH * n_sl

                        def tail_part(ch, e):
                            nonlocal mm_k
                            u_ch = u_sb[:, ch * GC : (ch + 1) * GC]
                            den = ch_pool.tile(
                                [128, GC, 4], f32, name="den", tag="den"
                            )
                            nc.vector.tensor_reduce(
                                den[:], e[:], axis=AX.X, op=alu.add
                            )
                            rec = ch_pool.tile(
                                [128, GC, 4], f32, name="rec", tag="rec"
                            )
                            nc.vector.reciprocal(rec[:], den[:])
                            c = ch_pool.tile(
                                [128, GC, 4, N], bf16, name="c", tag="c"
                            )
                            rec_b = (
                                rec[:]
                                .rearrange("p g i -> p g i ()")
                                .broadcast_to([128, GC, 4, N])
                            )
                            nc.vector.tensor_mul(c[:], e[:], rec_b)
                            prod = ch_pool.tile(
                                [128, GC, 4, O, N], bf16, name="prod", tag="pr"
                            )
                            c_b = (
                                c[:]
                                .rearrange("p g i n -> p g i () n")
                                .broadcast_to([128, GC, 4, O, N])
                            )
                            nc.vector.tensor_mul(prod[:], u_ch, c_b)
                            prod_f = prod[:].rearrange(
                                "p g i o n -> p (g i) o n"
                            )
                            for k in range(n_sl):
                                nc.tensor.matmul(
                                    s_ps[:, :, :, :],
                                    i4_sb[:],
                                    prod_f[:, SL * k : SL * (k + 1), :, :],
                                    start=(mm_k == 0),
                                    stop=(mm_k == n_mm - 1),
                                    skip_group_check=True,
                                )
                                mm_k += 1

                        pend = None
                        for ch in range(NCH):
                            u_ch = u_sb[:, ch * GC : (ch + 1) * GC]
                            vb_b = (
                                vb[:]
                                .rearrange("p o n -> p () () o n")
                                .broadcast_to([128, GC, 4, O, N])
                            )
                            p2 = ch_pool.tile(
                                [128, GC, 4, O, N], bf16, name="p2", tag="p2"
                            )
                            nc.vector.tensor_mul(p2[:], u_ch, vb_b)
                            # o-reduction on the PE: 16 identity-stationary
                            # matmuls accumulating into one PSUM tile.
                            agr_ps = agr_pool.tile(
                                [128, GC, 4, N], f32, name="agr_ps", tag="agr_ps"
                            )
                            for o in range(O):
                                nc.tensor.matmul(
                                    agr_ps[:],
                                    ii_sb[:],
                                    p2[:, :, :, o, :],
                                    start=(o == 0),
                                    stop=(o == O - 1),
                                    skip_group_check=True,
                                )
                            bl_sl = bl[:, ch * GC : (ch + 1) * GC]
                            e = ch_pool.tile(
                                [128, GC, 4, N], bf16, name="e", tag="e"
                            )
                            if first:
                                # bl_1 = agr_0 (kept for t=2); softmax exp
                                # straight from PSUM on ACT
                                nc.scalar.activation(bl_sl, agr_ps[:], act.Copy)
                                nc.scalar.activation(e[:], agr_ps[:], act.Exp)
                            else:
                                nc.vector.tensor_add(bl_sl, bl_sl, agr_ps[:])
                                nc.scalar.activation(e[:], bl_sl, act.Exp)
                            # software pipeline: previous chunk's softmax tail
                            # runs while this chunk's exp is on ACT
                            if pend is not None:
                                tail_part(*pend)
                            pend = (ch, e)
                        tail_part(*pend)
                        st = rpool.tile([BS, N, O], f32, name="st", tag="st")
                        nc.scalar.activation(
                            st[:],
                            s_ps[:, 0, :, :].rearrange("b o n -> b n o"),
                            act.Copy,
                        )
                        for k in range(1, SL):
                            nc.vector.tensor_add(
                                st[:],
                                st[:],
                                s_ps[:, k, :, :].rearrange("b o n -> b n o"),
                            )
                        v_sb, vb = squash_and_vb(st, t)
                        if t == niter - 1:
                            nc.sync.dma_start(
                                v_d[:], v_sb[:].rearrange("b n o -> b (n o)")
                            )

    nc.compile()
    return nc


def _prep_inputs(x, W):
    # xt[core][32j+c][G, i, b] = x[b0+b, 16G+4i+j, c]
    xr = x.reshape(NCORES, BS, NG, 4, 4, C)  # k, b, G, i, j, c
    xt = np.ascontiguousarray(
        xr.transpose(0, 4, 5, 2, 3, 1), dtype=BF16
    ).reshape(NCORES, 128, NG, 4, BS)
    # wt[ck][32j+c][Gck][i][n*16+o] = W[16G+4i+j, n, c, o], G = ck*WCH + Gck
    wr = W.reshape(NG, 4, 4, N, C, O)  # G, i, j, n, c, o
    wt = (
        np.ascontiguousarray(wr.transpose(2, 4, 0, 1, 3, 5), dtype=BF16)
        .reshape(128, NWCH, WCH, 4, NO)
        .transpose(1, 0, 2, 3, 4)
    )
    wt = np.ascontiguousarray(wt)
    i4 = np.ascontiguousarray(np.tile(np.eye(BS), (4, 1)), dtype=BF16)
    i32 = np.ascontiguousarray(np.tile(np.eye(BS), (1, 4)), dtype=BF16)
    ii = np.ascontiguousarray(np.eye(128), dtype=BF16)
    return xt, wt, i4, i32, ii


def kernel(x: np.ndarray, W: np.ndarray) -> np.ndarray:
    from concourse import bass_utils

    if "nc" not in _CACHE:
        _CACHE["nc"] = _build_program()
    nc = _CACHE["nc"]

    xt, wt, i4, i32, ii = _prep_inputs(
        np.asarray(x, np.float32), np.asarray(W, np.float32)
    )
    in_maps = [
        {
            "xt": np.ascontiguousarray(xt[k]),
            "wt": wt,
            "i4": i4,
            "i32": i32,
            "ii": ii,
        }
        for k in range(NCORES)
    ]
    import os

    trace = bool(int(os.environ.get("KERNEL_TRACE", "0")))
    res = bass_utils.run_bass_kernel_spmd(
        nc, in_maps, core_ids=list(range(NCORES)), trace=trace
    )
    if trace:
        _CACHE["last_results"] = res
        print(f"HW exec time: {res.exec_time_ns} ns")
        print(
            f"trace: {res.instructions_and_trace[1] if res.instructions_and_trace else None}"
        )
        print(f"profile_json: {res.profile_json}")
    out = np.concatenate(
        [res.results[k]["v"].reshape(BS, N, O) for k in range(NCORES)], axis=0
    )
    return out.astype(np.float32)


if __name__ == "__main__":
    x = np.random.randn(B, R, C).astype(np.float32)
    W = (np.random.randn(R, N, C, O) * 0.01).astype(np.float32)
    v = kernel(x, W)
    print("out", v.shape, v.dtype, float(np.abs(v).max()))

